# revision 28
# baseline (speedup 1.0000x reference)
"""AssociativeEmbeddingLoss on 8 TRN2 NeuronCores.

Reference, per image b (C=1, G=128 boxes):
    tl[g] = pred[b, 0, ty[g], tx[g]],  br[g] = target[b, 0, by[g], bx[g]]
    me = (tl + br) / 2
    pull_b = sum((tl-br)^2) / (2N)
    push_b = sum_{i != j} relu(1 - |me_i - me_j|) / (N*(N-1))
    out = (0.25 * sum_b pull_b, 0.25 * sum_b push_b)

Data-parallel over batch, 8 images per core. Only the 2*G*BP scalars the
loss touches are read from the big inputs, via 16 indirect DMAs (one
[128,1] column per image/tensor; the Q7 descriptor cost ~10ns/element is
the hard floor either way). Flat gather indices are computed on DVE in
f32 (exact below 2^24). Per-image compute (me, transpose, row-broadcast
matmul, |me_j - me_i|, and relu(1-|d|) = 1 - min(|d|,1) min+accumulate)
is pipelined behind the remaining gathers, so the post-gather tail is a
couple of tiny reductions. Each core emits its partial
[pull_sum, min_sum]; the host combines the 8 pairs (the unshard step).
"""

import numpy as np

import concourse.bacc as bacc
import concourse.bass as bass
import concourse.mybir as mybir
import concourse.tile as tile
from concourse.bass import IndirectOffsetOnAxis
from concourse.bass_utils import run_bass_kernel_spmd

B, C, H, W = 64, 1, 512, 512
G = 128                 # boxes per image; N = G*C = 128
N = G * C
NCORES = 8
BP = B // NCORES        # images per core
NPIX = BP * H * W
PULL_W, PUSH_W = 0.25, 0.25

F32 = mybir.dt.float32
I32 = mybir.dt.int32
AF = mybir.ActivationFunctionType
ALU = mybir.AluOpType

# |d| via DVE tensor_scalar op1=abs_max (else scalar-engine Abs activation)
USE_DVE_ABS = False


def _build_nc():
    nc = bacc.Bacc(
        "TRN2",
        target_bir_lowering=False,
        debug=False,
        enable_asserts=False,
        num_devices=NCORES,
    )
    pred = nc.dram_tensor("pred", [NPIX, 1], F32, kind="ExternalInput")
    targ = nc.dram_tensor("target", [NPIX, 1], F32, kind="ExternalInput")
    match = nc.dram_tensor("match", [BP, G * 4], F32, kind="ExternalInput")
    # consts: [:, 0:128] identity, [:, 128:136] base_row (b*H), [:, 136] ones
    consts = nc.dram_tensor("consts", [G, 2 * G + BP + 1], F32, kind="ExternalInput")
    out = nc.dram_tensor("out", [1, 2], F32, kind="ExternalOutput")

    with tile.TileContext(nc) as tc:
        _kernel_body(nc, tc, pred, targ, match, consts, out)
    nc.compile()
    return nc


def _kernel_body(nc, tc, pred, targ, match, consts, out):
    with (
        tc.tile_pool(name="sb", bufs=1) as sb,
        tc.tile_pool(name="ps", bufs=1, space="PSUM") as ps,
        tc.tile_pool(name="psr", bufs=2, space="PSUM") as psr,
    ):
        # ---- coords first: [128, (b, c)] f32, partition = g; two DMAs so the
        # 16B-granule transfers run on two queues ----
        cC = sb.tile([G, BP * 4], F32, tag="cC")
        half = BP // 2
        srcC0 = bass.AP(match.ap().tensor, 0, [[4, G], [G * 4, half], [1, 4]])
        srcC1 = bass.AP(match.ap().tensor, G * 4 * half,
                        [[4, G], [G * 4, half], [1, 4]])
        cr = cC[:].rearrange("g (b c) -> g b c", b=BP, c=4)
        nc.sync.dma_start(out=cr[:, 0:half, :], in_=srcC0)
        nc.sync.dma_start(out=cr[:, half:BP, :], in_=srcC1)

        ct = sb.tile([G, 2 * G + BP + 1], F32, tag="ct")
        nc.sync.dma_start(out=ct[:], in_=consts.ap())
        ident = ct[:, 0:G]
        ones = ct[:, G + BP : G + BP + 1]     # [128, 1]
        ones_row = ct[0:1, G + BP + 1 : G + BP + 1 + G]   # [1, 128]

        # ---- flat in-image indices: idx = y * W + x  (f32 exact) ----
        # the per-image b*H*W base goes in via indirect-DMA element_offset
        def flatidx(name, ysel, xsel):
            f = sb.tile([G, BP], F32, tag=name + "_f")
            i = sb.tile([G, BP], I32, tag=name)
            nc.vector.tensor_scalar(
                out=f[:], in0=cr[:, :, ysel], scalar1=float(W), scalar2=None,
                op0=ALU.mult,
            )
            nc.vector.tensor_tensor(out=f[:], in0=f[:], in1=cr[:, :, xsel], op=ALU.add)
            nc.vector.tensor_copy(out=i[:], in_=f[:])
            return i

        tl_idx = flatidx("tlidx", 0, 1)
        br_idx = flatidx("bridx", 2, 3)

        # ---- gathers + per-image pipeline ----
        me2c = sb.tile([G, BP], F32, tag="me2c")
        negme = sb.tile([G, BP], F32, tag="negme")
        dsub = sb.tile([G, BP], F32, tag="dsub")
        min_cols = sb.tile([G, BP], F32, tag="min_cols")
        absd = None
        if not USE_DVE_ABS:
            absd = sb.tile([G, BP * G], F32, tag="absd")

        for b in range(BP):
            cs = slice(b, b + 1)
            # separate tiles per image so gather b+1 has no false WAR on
            # the compute reading image b
            tlb = sb.tile([G, 1], F32, tag=f"tl{b}")
            brb = sb.tile([G, 1], F32, tag=f"br{b}")
            nc.gpsimd.indirect_dma_start(
                out=tlb[:], out_offset=None, in_=pred.ap(),
                in_offset=IndirectOffsetOnAxis(ap=tl_idx[:, cs], axis=0),
                element_offset=b * H * W,
            )
            nc.gpsimd.indirect_dma_start(
                out=brb[:], out_offset=None, in_=targ.ap(),
                in_offset=IndirectOffsetOnAxis(ap=br_idx[:, cs], axis=0),
                element_offset=b * H * W,
            )
            # per-image compute, overlapping the remaining gathers
            nc.vector.tensor_sub(dsub[:, cs], tlb[:], brb[:])
            nc.vector.tensor_add(me2c[:, cs], tlb[:], brb[:])
            nc.vector.tensor_scalar(
                out=negme[:, cs], in0=me2c[:, cs], scalar1=-0.5, scalar2=None,
                op0=ALU.mult,
            )
            # me row: transpose the column, scale 0.5 on the copy out of PSUM
            rowp = psr.tile([1, G], F32, tag="rowp")
            nc.tensor.transpose(out=rowp[:], in_=me2c[:, cs], identity=ident)
            merow = sb.tile([1, G], F32, tag=f"merow{b % 2}")
            nc.vector.tensor_scalar(
                out=merow[:], in0=rowp[:], scalar1=0.5, scalar2=None, op0=ALU.mult,
            )
            # R[i, j] = me[b, j]
            Rp = psr.tile([G, G], F32, tag="Rp")
            nc.tensor.matmul(
                out=Rp[:], lhsT=ones_row, rhs=merow[:], start=True, stop=True,
            )
            if USE_DVE_ABS:
                # |R - me_i| then min(.,1), accumulated along j
                ad = sb.tile([G, G], F32, tag=f"ad{b % 2}")
                nc.vector.tensor_scalar(
                    out=ad[:], in0=Rp[:], scalar1=negme[:, cs], scalar2=0.0,
                    op0=ALU.add, op1=ALU.abs_max,
                )
                nc.vector.tensor_scalar(
                    out=ad[:], in0=ad[:], scalar1=1.0, scalar2=0.0,
                    op0=ALU.min, op1=ALU.add, accum_out=min_cols[:, cs],
                )
            else:
                nc.scalar.activation(
                    out=absd[:, b * G : (b + 1) * G], in_=Rp[:],
                    func=AF.Abs, bias=negme[:, cs], scale=1.0,
                )
                nc.vector.tensor_scalar(
                    out=absd[:, b * G : (b + 1) * G],
                    in0=absd[:, b * G : (b + 1) * G], scalar1=1.0, scalar2=0.0,
                    op0=ALU.min, op1=ALU.add, accum_out=min_cols[:, cs],
                )

        # ---- tail reductions ----
        sq = sb.tile([G, BP], F32, tag="sq")
        nc.vector.tensor_mul(sq[:], dsub[:], dsub[:])
        pull_col = sb.tile([G, 1], F32, tag="pull_col")
        nc.vector.tensor_reduce(
            out=pull_col[:], in_=sq[:], op=ALU.add, axis=mybir.AxisListType.X,
        )
        min_col = sb.tile([G, 1], F32, tag="min_col")
        nc.vector.tensor_reduce(
            out=min_col[:], in_=min_cols[:], op=ALU.add, axis=mybir.AxisListType.X,
        )
        fin = ps.tile([1, 2], F32, tag="fin")
        nc.tensor.matmul(out=fin[0:1, 0:1], lhsT=pull_col[:], rhs=ones,
                         start=True, stop=True)
        nc.tensor.matmul(out=fin[0:1, 1:2], lhsT=min_col[:], rhs=ones,
                         start=True, stop=True)
        # pull = c_pull * S; push = (BP*N*(N-1) - minsum) * c_push
        c_pull = PULL_W / (2.0 * N)
        c_push = PUSH_W / (N * (N - 1))
        res = sb.tile([1, 2], F32, tag="res")
        nc.scalar.activation(out=res[0:1, 0:1], in_=fin[0:1, 0:1], func=AF.Copy,
                             scale=c_pull)
        nc.scalar.activation(out=res[0:1, 1:2], in_=fin[0:1, 1:2], func=AF.Copy,
                             scale=-c_push, bias=float(BP * N * (N - 1)) * c_push)
        nc.sync.dma_start(out=out.ap(), in_=res[:])


_NC_CACHE = None


def _get_nc():
    global _NC_CACHE
    if _NC_CACHE is None:
        _NC_CACHE = _build_nc()
    return _NC_CACHE


def _consts():
    c = np.zeros((G, 2 * G + BP + 1), dtype=np.float32)
    c[:, 0:G] = np.eye(G, dtype=np.float32)
    c[:, G : G + BP] = (np.arange(BP, dtype=np.float32) * H)[None, :]
    c[:, G + BP] = 1.0
    c[0, G + BP + 1 :] = 1.0
    return c


def make_in_maps(pred, target, match):
    pred = np.asarray(pred, dtype=np.float32)
    target = np.asarray(target, dtype=np.float32)
    match = np.asarray(match)
    consts = _consts()
    in_maps = []
    for k in range(NCORES):
        sl = slice(k * BP, (k + 1) * BP)
        in_maps.append({
            "pred": np.ascontiguousarray(pred[sl]).reshape(NPIX, 1),
            "target": np.ascontiguousarray(target[sl]).reshape(NPIX, 1),
            "match": np.ascontiguousarray(match[sl]).astype(np.float32).reshape(BP, G * 4),
            "consts": consts,
        })
    return in_maps


def kernel(pred, target, match, _trace=False):
    nc = _get_nc()
    in_maps = make_in_maps(pred, target, match)
    res = run_bass_kernel_spmd(nc, in_maps, core_ids=list(range(NCORES)), trace=_trace)
    total = np.zeros((1, 2), dtype=np.float64)
    for r in res.results:
        total += r["out"].astype(np.float64)
    out = (np.float32(total[0, 0]), np.float32(total[0, 1]))
    if _trace:
        return out, res
    return out


# revision 31
# speedup vs baseline: 1.0014x; 1.0014x over previous
"""AssociativeEmbeddingLoss on 8 TRN2 NeuronCores.

Reference, per image b (C=1, G=128 boxes):
    tl[g] = pred[b, 0, ty[g], tx[g]],  br[g] = target[b, 0, by[g], bx[g]]
    me = (tl + br) / 2
    pull_b = sum((tl-br)^2) / (2N)
    push_b = sum_{i != j} relu(1 - |me_i - me_j|) / (N*(N-1))
    out = (0.25 * sum_b pull_b, 0.25 * sum_b push_b)

Data-parallel over batch, 8 images per core. Only the 2*G*BP scalars the
loss touches are read from the big inputs, via 16 indirect DMAs (one
[128,1] column per image/tensor; the Q7 descriptor cost ~10ns/element is
the hard floor either way). Flat gather indices are computed on DVE in
f32 (exact below 2^24). Per-image compute (me, transpose, row-broadcast
matmul, |me_j - me_i|, and relu(1-|d|) = 1 - min(|d|,1) min+accumulate)
is pipelined behind the remaining gathers, so the post-gather tail is a
couple of tiny reductions. Each core emits its partial
[pull_sum, min_sum]; the host combines the 8 pairs (the unshard step).
"""

import numpy as np

import concourse.bacc as bacc
import concourse.bass as bass
import concourse.mybir as mybir
import concourse.tile as tile
from concourse.bass import IndirectOffsetOnAxis
from concourse.bass_utils import run_bass_kernel_spmd

B, C, H, W = 64, 1, 512, 512
G = 128                 # boxes per image; N = G*C = 128
N = G * C
NCORES = 8
BP = B // NCORES        # images per core
NPIX = BP * H * W
PULL_W, PUSH_W = 0.25, 0.25

F32 = mybir.dt.float32
I32 = mybir.dt.int32
AF = mybir.ActivationFunctionType
ALU = mybir.AluOpType

# |d| via DVE tensor_scalar op1=abs_max (else scalar-engine Abs activation)
USE_DVE_ABS = False


def _build_nc():
    nc = bacc.Bacc(
        "TRN2",
        target_bir_lowering=False,
        debug=False,
        enable_asserts=False,
        num_devices=NCORES,
    )
    pred = nc.dram_tensor("pred", [NPIX, 1], F32, kind="ExternalInput")
    targ = nc.dram_tensor("target", [NPIX, 1], F32, kind="ExternalInput")
    match = nc.dram_tensor("match", [BP, G * 4], F32, kind="ExternalInput")
    # consts: [:, 0:128] identity, [:, 128:136] base_row (b*H), [:, 136] ones
    consts = nc.dram_tensor("consts", [G, 2 * G + BP + 1], F32, kind="ExternalInput")
    out = nc.dram_tensor("out", [1, 2], F32, kind="ExternalOutput")

    with tile.TileContext(nc) as tc:
        _kernel_body(nc, tc, pred, targ, match, consts, out)
    nc.compile()
    return nc


def _kernel_body(nc, tc, pred, targ, match, consts, out):
    with (
        tc.tile_pool(name="sb", bufs=1) as sb,
        tc.tile_pool(name="ps", bufs=1, space="PSUM") as ps,
        tc.tile_pool(name="psr", bufs=2, space="PSUM") as psr,
    ):
        # ---- consts + contiguous match load ----
        ct = sb.tile([G, 2 * G + BP + 1], F32, tag="ct")
        nc.sync.dma_start(out=ct[:], in_=consts.ap())
        ident = ct[:, 0:G]
        base_row = ct[:, G : G + BP]          # [128, BP] value b*H
        ones = ct[:, G + BP : G + BP + 1]     # [128, 1]
        ones_row = ct[0:1, G + BP + 1 : G + BP + 1 + G]   # [1, 128]

        t8 = sb.tile([BP, G * 4], F32, tag="t8")
        nc.sync.dma_start(out=t8[:], in_=match.ap())
        t8v = t8[:].rearrange("b (g c) -> b g c", g=G, c=4)

        # coords to [128(g), 8(b)] via PE transposes (beats a 16B-granule
        # strided DMA on transfer time)
        def coordT(name, csel):
            p = psr.tile([G, BP], F32, tag="coordp")
            nc.tensor.transpose(out=p[:], in_=t8v[:, :, csel], identity=ident[0:BP, 0:BP])
            return p

        # ---- flat indices: idx = (y + b*H) * W + x  (f32 exact) ----
        def flatidx(name, ysel, xsel):
            yp = coordT(name + "_y", ysel)
            xp = coordT(name + "_x", xsel)
            f = sb.tile([G, BP], F32, tag=name + "_f")
            i = sb.tile([G, BP], I32, tag=name)
            nc.vector.tensor_tensor(out=f[:], in0=yp[:], in1=base_row, op=ALU.add)
            nc.vector.tensor_scalar(
                out=f[:], in0=f[:], scalar1=float(W), scalar2=None, op0=ALU.mult,
            )
            nc.vector.tensor_tensor(out=f[:], in0=f[:], in1=xp[:], op=ALU.add)
            nc.vector.tensor_copy(out=i[:], in_=f[:])
            return i

        tl_idx = flatidx("tlidx", 0, 1)
        br_idx = flatidx("bridx", 2, 3)

        # ---- gathers + per-image pipeline ----
        tl = sb.tile([G, BP], F32, tag="tl")
        br = sb.tile([G, BP], F32, tag="br")
        me2c = sb.tile([G, BP], F32, tag="me2c")
        negme = sb.tile([G, BP], F32, tag="negme")
        dsub = sb.tile([G, BP], F32, tag="dsub")
        min_cols = sb.tile([G, BP], F32, tag="min_cols")
        absd = None
        if not USE_DVE_ABS:
            absd = sb.tile([G, BP * G], F32, tag="absd")

        for b in range(BP):
            cs = slice(b, b + 1)
            nc.gpsimd.indirect_dma_start(
                out=tl[:, cs], out_offset=None, in_=pred.ap(),
                in_offset=IndirectOffsetOnAxis(ap=tl_idx[:, cs], axis=0),
            )
            nc.gpsimd.indirect_dma_start(
                out=br[:, cs], out_offset=None, in_=targ.ap(),
                in_offset=IndirectOffsetOnAxis(ap=br_idx[:, cs], axis=0),
            )
            # per-image compute, overlapping the remaining gathers
            nc.vector.tensor_sub(dsub[:, cs], tl[:, cs], br[:, cs])
            nc.vector.tensor_add(me2c[:, cs], tl[:, cs], br[:, cs])
            nc.vector.tensor_scalar(
                out=negme[:, cs], in0=me2c[:, cs], scalar1=-0.5, scalar2=None,
                op0=ALU.mult,
            )
            # me row: transpose the column, scale 0.5 on the copy out of PSUM
            rowp = psr.tile([1, G], F32, tag="rowp")
            nc.tensor.transpose(out=rowp[:], in_=me2c[:, cs], identity=ident)
            merow = sb.tile([1, G], F32, tag=f"merow{b % 2}")
            nc.vector.tensor_scalar(
                out=merow[:], in0=rowp[:], scalar1=0.5, scalar2=None, op0=ALU.mult,
            )
            # R[i, j] = me[b, j]
            Rp = psr.tile([G, G], F32, tag="Rp")
            nc.tensor.matmul(
                out=Rp[:], lhsT=ones_row, rhs=merow[:], start=True, stop=True,
            )
            if USE_DVE_ABS:
                # |R - me_i| then min(.,1), accumulated along j
                ad = sb.tile([G, G], F32, tag=f"ad{b % 2}")
                nc.vector.tensor_scalar(
                    out=ad[:], in0=Rp[:], scalar1=negme[:, cs], scalar2=0.0,
                    op0=ALU.add, op1=ALU.abs_max,
                )
                nc.vector.tensor_scalar(
                    out=ad[:], in0=ad[:], scalar1=1.0, scalar2=0.0,
                    op0=ALU.min, op1=ALU.add, accum_out=min_cols[:, cs],
                )
            else:
                nc.scalar.activation(
                    out=absd[:, b * G : (b + 1) * G], in_=Rp[:],
                    func=AF.Abs, bias=negme[:, cs], scale=1.0,
                )
                nc.vector.tensor_scalar(
                    out=absd[:, b * G : (b + 1) * G],
                    in0=absd[:, b * G : (b + 1) * G], scalar1=1.0, scalar2=0.0,
                    op0=ALU.min, op1=ALU.add, accum_out=min_cols[:, cs],
                )

        # ---- tail reductions ----
        sq = sb.tile([G, BP], F32, tag="sq")
        nc.vector.tensor_mul(sq[:], dsub[:], dsub[:])
        pull_col = sb.tile([G, 1], F32, tag="pull_col")
        nc.vector.tensor_reduce(
            out=pull_col[:], in_=sq[:], op=ALU.add, axis=mybir.AxisListType.X,
        )
        min_col = sb.tile([G, 1], F32, tag="min_col")
        nc.vector.tensor_reduce(
            out=min_col[:], in_=min_cols[:], op=ALU.add, axis=mybir.AxisListType.X,
        )
        fin = ps.tile([1, 2], F32, tag="fin")
        nc.tensor.matmul(out=fin[0:1, 0:1], lhsT=pull_col[:], rhs=ones,
                         start=True, stop=True)
        nc.tensor.matmul(out=fin[0:1, 1:2], lhsT=min_col[:], rhs=ones,
                         start=True, stop=True)
        # pull = c_pull * S; push = (BP*N*(N-1) - minsum) * c_push
        c_pull = PULL_W / (2.0 * N)
        c_push = PUSH_W / (N * (N - 1))
        res = sb.tile([1, 2], F32, tag="res")
        nc.scalar.activation(out=res[0:1, 0:1], in_=fin[0:1, 0:1], func=AF.Copy,
                             scale=c_pull)
        nc.scalar.activation(out=res[0:1, 1:2], in_=fin[0:1, 1:2], func=AF.Copy,
                             scale=-c_push, bias=float(BP * N * (N - 1)) * c_push)
        nc.sync.dma_start(out=out.ap(), in_=res[:])


_NC_CACHE = None


def _get_nc():
    global _NC_CACHE
    if _NC_CACHE is None:
        _NC_CACHE = _build_nc()
    return _NC_CACHE


def _consts():
    c = np.zeros((G, 2 * G + BP + 1), dtype=np.float32)
    c[:, 0:G] = np.eye(G, dtype=np.float32)
    c[:, G : G + BP] = (np.arange(BP, dtype=np.float32) * H)[None, :]
    c[:, G + BP] = 1.0
    c[0, G + BP + 1 :] = 1.0
    return c


def make_in_maps(pred, target, match):
    pred = np.asarray(pred, dtype=np.float32)
    target = np.asarray(target, dtype=np.float32)
    match = np.asarray(match)
    consts = _consts()
    in_maps = []
    for k in range(NCORES):
        sl = slice(k * BP, (k + 1) * BP)
        in_maps.append({
            "pred": np.ascontiguousarray(pred[sl]).reshape(NPIX, 1),
            "target": np.ascontiguousarray(target[sl]).reshape(NPIX, 1),
            "match": np.ascontiguousarray(match[sl]).astype(np.float32).reshape(BP, G * 4),
            "consts": consts,
        })
    return in_maps


def kernel(pred, target, match, _trace=False):
    nc = _get_nc()
    in_maps = make_in_maps(pred, target, match)
    res = run_bass_kernel_spmd(nc, in_maps, core_ids=list(range(NCORES)), trace=_trace)
    total = np.zeros((1, 2), dtype=np.float64)
    for r in res.results:
        total += r["out"].astype(np.float64)
    out = (np.float32(total[0, 0]), np.float32(total[0, 1]))
    if _trace:
        return out, res
    return out
